# revision 36
# baseline (speedup 1.0000x reference)
"""Trainium2 Bass kernel for single-head cross-attention with additive mask.

Computation (matches the reference):
    q = tgt @ wq + bq
    k = src @ wk (+ bk dropped: softmax cancels a per-row constant exactly)
    v = src @ wv (bv applied on host in the epilogue)
    s = (q k^T + mask) / sqrt(DQ)
    out = softmax(s) @ v + bv

Two SPMD launches on 8 cores (all matmul inputs fp16, fp32 PSUM accum):
  L1: each core projects K and V for 1/8 of the global (B*S) src rows in a
      single fused matmul (wk|wv concatenated -> kvT [128, 2048] fp16 out),
      pipelined per 512-row quarter (DMA -> 8 matmuls -> evac -> store).
  host: pure layout glue -- assembles kt2 (d-major K) and v65 (V with an
      appended ones column for the softmax denominator).
  L2: tgt sharded 8 ways; core c handles tgt rows [c*512,(c+1)*512) of every
      batch so its mask slice is read from HBM exactly once.

L2 computes scores transposed (src-block on PSUM partitions) batch-column
outer: column c processes batches {c, c+2}.  exp((qk+mask)/8) is split
across three engines to clear the scalar-engine bottleneck (ACT is 1
elem/lane/cycle, a 55us floor if it ran every exp alone):
  - ACT_SGS: mask enters PSUM ahead of QK via an identity-weight matmul,
    ACT reads (qk+mask) from PSUM and emits fp16 exp at scale=1/8;
  - GP_SGS: ACT emits exp(qk/8), the otherwise-idle GPSIMD multiplies by
    host-baked exp(mask/8) (placed early in the loop -- longest chain);
  - DVE_SGS: the vector engine computes a Schraudolph bit-trick exp in the
    fp16 bit domain: bits16 = int16(qk*A16 + bmask) where bmask (host
    int16) carries mask*A16 + (15-sigma)*1024; bitcast to fp16 IS the
    approximate exp (rel err ~3%, diluted to ~1.2e-2 end-to-end).
PV matmul emission trails QK by PIPE_LAG blocks so the tensor engine never
idles waiting for an exp (keeps HAM un-throttled; the PE is the bottleneck
at its 16-bit streaming roofline).  All DMA rides the two HW-DGE engines
(sync/scalar) as host-prearranged contiguous SBUF images, issued in
need-order so the q projection's tgt slices land first.  PV accumulates
fp32 in PSUM with a 65th "ones" output row; the final division by the
softmax denominator (+bv) runs on the host.
"""
import numpy as np

B, S, D, DQ = 4, 4096, 1024, 64
NCORES = 8
TS = S // NCORES            # 512 tgt rows per core
SR = (B * S) // NCORES      # 2048 global src rows per core (L1)
SB = S // 128               # 32 src blocks per batch
GK = B * SB                 # 128 global src blocks
CORES = list(range(NCORES))
F32 = np.float32
FP16 = np.float16
PIPE_LAG = 5

# --- per-src-block exp-path assignment ---
# S: DVE Schraudolph bit-trick (mask folded into the int16 affine bias)
# G: exact ACT exp, then GPSIMD multiply by host-baked exp(mask/8)
# V: exact ACT exp, then DVE multiply by exp(mask/8)
# M: mask into PSUM via identity matmul, then exact ACT exp
N_DVE_SG = 14
SIGMA = 0.035
A16 = (2.0 ** 10) * np.log2(np.e) / 8.0
B16C = (2.0 ** 10) * (15.0 - SIGMA)
# GP blocks early (their exp->gpsimd-multiply chain is the longest, so it
# must not gate the end of a column); mask-path blocks last (shortest
# non-PE chain); Schraudolph fills the rest.
GP_SGS = (0, 3, 6, 9, 12, 15, 18, 21)
ACT_SGS = (5, 13, 17, 25, 26, 27, 28, 29, 30, 31)
V2_SGS = ()
DVE_SGS = tuple(g for g in range(SB)
                if g not in GP_SGS and g not in ACT_SGS)     # 14 blocks
EM_SGS = tuple(sorted(GP_SGS + V2_SGS))                      # share emask
N_ASG, N_DSG, N_GSG = len(ACT_SGS), len(DVE_SGS), len(EM_SGS)
# position of each sg within its path's packed mask array
_MASK_IDX = {g: i for i, g in enumerate(ACT_SGS)}
_MASK_IDX.update({g: i for i, g in enumerate(DVE_SGS)})
_MASK_IDX.update({g: i for i, g in enumerate(EM_SGS)})

_CACHE = {}


def _build_l1():
    import concourse.mybir as mybir
    import concourse.tile as tile
    from concourse import bacc

    f32 = mybir.dt.float32
    fp16 = mybir.dt.float16

    nc = bacc.Bacc("TRN2", target_bir_lowering=False, debug=False,
                   num_devices=NCORES)
    # host-prearranged SBUF image per src-row quarter: [q][p, (j, t)]
    srcq = nc.dram_tensor("srcq", [4, 128, 8 * 512], fp16,
                          kind="ExternalInput")
    wkv = nc.dram_tensor("wkv", [D, 128], fp16, kind="ExternalInput")
    kvT = nc.dram_tensor("kvT", [128, SR], fp16, kind="ExternalOutput")

    with tile.TileContext(nc) as tc:
        with (
            tc.tile_pool(name="const", bufs=1) as constp,
            tc.tile_pool(name="big", bufs=1) as bigp,
            tc.tile_pool(name="stream", bufs=2) as streamp,
            tc.tile_pool(name="pp", bufs=1, space="PSUM") as pp,
        ):
            wkv_sb = constp.tile([128, 8 * 128], fp16)
            nc.sync.dma_start(
                out=wkv_sb.rearrange("p (j m) -> p j m", m=128),
                in_=wkv.rearrange("(j p) m -> p j m", p=128))
            sts = []
            for q in range(4):
                st = streamp.tile([128, 8 * 512], fp16, tag="xs", bufs=4,
                                  name=f"st{q}")
                eng = nc.sync if q % 2 == 0 else nc.scalar
                eng.dma_start(out=st[:], in_=srcq[q])
                sts.append(st)
            kv_sb = bigp.tile([128, SR], fp16)
            # PE warm-up during the src DMA wait (see L2 comment)
            warm = pp.tile([128, 512], f32, tag="kv", bufs=2, name="warm")
            for _ in range(10):
                nc.tensor.matmul(warm[:], lhsT=wkv_sb[:, 0:128],
                                 rhs=wkv_sb[:, 0:512], start=True, stop=True)
            # per-quarter pipeline: 8 matmuls -> evac -> store, while the
            # next quarter's src image is still in flight
            for q in range(4):
                kv_ps = pp.tile([128, 512], f32, tag="kv", bufs=2,
                                name=f"kv_ps{q}")
                for j in range(8):
                    nc.tensor.matmul(
                        kv_ps[:],
                        lhsT=wkv_sb[:, j * 128:(j + 1) * 128],
                        rhs=sts[q][:, j * 512:(j + 1) * 512],
                        start=(j == 0), stop=(j == 7))
                if q % 2 == 0:
                    nc.scalar.copy(kv_sb[:, q * 512:(q + 1) * 512],
                                   kv_ps[:])
                else:
                    nc.vector.tensor_copy(kv_sb[:, q * 512:(q + 1) * 512],
                                          kv_ps[:])
                eng = nc.sync if q % 2 == 0 else nc.scalar
                eng.dma_start(out=kvT[:, q * 512:(q + 1) * 512],
                              in_=kv_sb[:, q * 512:(q + 1) * 512])
    nc.compile()
    return nc


def _build_l2():
    import concourse.mybir as mybir
    import concourse.tile as tile
    from concourse import bacc
    from concourse.masks import make_identity

    f32 = mybir.dt.float32
    fp16 = mybir.dt.float16
    i16 = mybir.dt.int16
    AF = mybir.ActivationFunctionType
    ALU = mybir.AluOpType

    nc = bacc.Bacc("TRN2", target_bir_lowering=False, debug=False,
                   num_devices=NCORES)
    # kt2: partitions 0-63 = kT of batches 0|1; 64-127 = batches 2|3
    kt2d = nc.dram_tensor("kt2", [128, 2 * S], fp16, kind="ExternalInput")
    # v65, batch-column order (b0, b2, b1, b3): row p, col (bb, kg', c):
    # element = v[b(bb), kg'*128 + p, c] | ones
    v65d = nc.dram_tensor("v65", [128, GK * (DQ + 1)], fp16,
                          kind="ExternalInput")
    # host-prearranged SBUF image: tgt[b][p, (half, j, t)] (contiguous rows)
    tgtd = nc.dram_tensor("tgt", [B, 128, 8 * TS], fp16,
                          kind="ExternalInput")
    # packed SBUF-image mask rows for the ACT-path src blocks (fp16)
    if N_ASG:
        maskA = nc.dram_tensor("maskA", [128, N_ASG * TS], fp16,
                               kind="ExternalInput")
    # packed Schraudolph bias image for the DVE-path src blocks (int16)
    if N_DSG:
        bmaskd = nc.dram_tensor("bmask", [128, N_DSG * TS], i16,
                                kind="ExternalInput")
    # packed exp(mask/8) image for the GPSIMD-multiply src blocks (fp16)
    if N_GSG:
        emaskd = nc.dram_tensor("emask", [128, N_GSG * TS], fp16,
                                kind="ExternalInput")
    wq = nc.dram_tensor("wq", [D, DQ], fp16, kind="ExternalInput")
    bq = nc.dram_tensor("bq", [DQ], f32, kind="ExternalInput")
    # out rows 0-63: (attn @ v)^T numerator; row 64: softmax denominator
    o = nc.dram_tensor("o", [B, DQ + 1, TS], f32, kind="ExternalOutput")

    with tile.TileContext(nc) as tc:
        with (
            tc.tile_pool(name="const", bufs=1) as constp,
            tc.tile_pool(name="big", bufs=1) as bigp,
            tc.tile_pool(name="stream", bufs=2) as streamp,
            tc.tile_pool(name="pp", bufs=1, space="PSUM") as pp,
        ):
            # ---- constants + all input DMA, issued in need-order
            wq_sb = constp.tile([128, 8 * DQ], fp16)
            bq_sb = constp.tile([128, 1], f32)
            ident = constp.tile([128, 128], fp16)
            make_identity(nc, ident[:])

            kT2 = bigp.tile([128, 2 * S], fp16)
            v2 = bigp.tile([128, GK * (DQ + 1)], fp16)
            VQ = 32 * (DQ + 1)
            if N_ASG:
                maskA_sb = bigp.tile([128, N_ASG * TS], fp16)
            if N_DSG:
                bmask_sb = bigp.tile([128, N_DSG * TS], i16)
            if N_GSG:
                emask_sb = bigp.tile([128, N_GSG * TS], fp16)
            tgs = {}
            for b in (0, 2, 1, 3):
                tgs[b] = streamp.tile([128, 8 * TS], fp16, tag="tg", bufs=4,
                                      name=f"tg{b}")

            def load_tg(b, eng):
                eng.dma_start(out=tgs[b][:], in_=tgtd[b])

            def load_kt2(lo, hi, eng):
                eng.dma_start(out=kT2[:, lo:hi], in_=kt2d[:, lo:hi])

            def load_v2(i0, eng):
                eng.dma_start(out=v2[:, i0 * VQ:(i0 + 2) * VQ],
                              in_=v65d[:, i0 * VQ:(i0 + 2) * VQ])

            def load_img(sb_tile, dram, lo, hi, n, eng):
                hi = min(hi, n)
                if n and lo < hi:
                    eng.dma_start(out=sb_tile[:, lo * TS:hi * TS],
                                  in_=dram[:, lo * TS:hi * TS])

            # need-ordered issue, split across the two HW-DGE sequencers;
            # tiny first chunks so sg 0 unblocks as early as possible
            nc.scalar.dma_start(
                out=wq_sb.rearrange("p (j m) -> p j m", m=DQ),
                in_=wq.rearrange("(j p) m -> p j m", p=128))
            load_tg(0, nc.sync)
            load_tg(2, nc.scalar)
            load_kt2(0, 512, nc.sync)       # column 0, sg 0-3
            nc.sync.dma_start(out=bq_sb[0:64, :],
                              in_=bq.rearrange("(p o) -> p o", o=1))
            nc.sync.dma_start(out=bq_sb[64:128, :],
                              in_=bq.rearrange("(p o) -> p o", o=1))
            if N_DSG:
                load_img(bmask_sb, bmaskd, 0, 3, N_DSG, nc.scalar)
            if N_GSG:
                load_img(emask_sb, emaskd, 0, 3, N_GSG, nc.scalar)
            if N_ASG:
                load_img(maskA_sb, maskA, 0, N_ASG, N_ASG, nc.scalar)
            load_kt2(512, 2048, nc.scalar)  # column 0, sg 4-15
            if N_DSG:
                load_img(bmask_sb, bmaskd, 3, 8, N_DSG, nc.sync)
            if N_GSG:
                load_img(emask_sb, emaskd, 3, 8, N_GSG, nc.scalar)
            load_v2(0, nc.sync)             # column-0 batches (b0, b2)
            if N_DSG:
                load_img(bmask_sb, bmaskd, 8, N_DSG, N_DSG, nc.sync)
            if N_GSG:
                load_img(emask_sb, emaskd, 8, N_GSG, N_GSG, nc.scalar)
            load_kt2(2048, 4096, nc.scalar)  # column 0, sg 16-31
            load_tg(1, nc.sync)
            load_tg(3, nc.sync)
            load_kt2(4096, 8192, nc.scalar)  # column 1
            load_v2(2, nc.sync)             # column-1 batches (b1, b3)

            # PE warm-up: the HAM clock gate needs ~3.4us of sustained matmul
            # activity to lift the 1.2->2.4 GHz throttle.  The PE is idle
            # while the first tgt/kt2 slices stream in, so burn that window
            # on dummy matmuls over the (tiny, already-landed) wq tile; the
            # q projection and the first score blocks then run at full clock.
            warm = pp.tile([128, TS], f32, tag="qk", bufs=3, name="warm")
            for _ in range(16):
                nc.tensor.matmul(warm[:], lhsT=wq_sb[:, 0:128],
                                 rhs=wq_sb[:, 0:512], start=True, stop=True)

            # ---- q projection (column order; b1/b3 emitted mid-loop below)
            qT_sb = bigp.tile([128, 2 * TS], fp16)

            def qproj(b):
                pb, colb = (b // 2) * 64, (b % 2) * TS
                q_ps = pp.tile([128, TS], f32, tag="qk", bufs=3,
                               name=f"q_ps{b}")
                for j in range(8):
                    nc.tensor.matmul(
                        q_ps[pb:pb + 64, :],
                        lhsT=wq_sb[:, j * DQ:(j + 1) * DQ],
                        rhs=tgs[b][:, j * TS:(j + 1) * TS],
                        start=(j == 0), stop=(j == 7))
                nc.scalar.activation(
                    qT_sb[pb:pb + 64, colb:colb + TS], q_ps[pb:pb + 64, :],
                    AF.Identity, bias=bq_sb[pb:pb + 64, :])

            qproj(0)
            qproj(2)

            # ---- attention main loop: batch-column outer; column c handles
            # batches {c, c+2} on disjoint 64-row PE tiles.  PV emission
            # trails by PIPE_LAG so the PE never waits on an exp.
            for col in range(2):
                pv = [pp.tile([DQ + 1, TS], f32, tag=f"pv{h}",
                              name=f"pv{col}_{h}") for h in range(2)]
                pts = {}

                def emit_qk_exp(sg, col=col, pts=pts):
                    dve = sg in DVE_SGS
                    em = sg in EM_SGS
                    mi = _MASK_IDX[sg]
                    qkt = pp.tile([128, 2 * TS], f32, tag="qk", bufs=3,
                                  name=f"qkt{col}_{sg}")
                    if not (dve or em):
                        for half in range(2):
                            nc.tensor.matmul(
                                qkt[:, half * TS:(half + 1) * TS],
                                lhsT=ident[:],
                                rhs=maskA_sb[:, mi * TS:(mi + 1) * TS],
                                start=True, stop=False)
                    for half in range(2):
                        nc.tensor.matmul(
                            qkt[:, half * TS:(half + 1) * TS],
                            lhsT=kT2[half * 64:half * 64 + 64,
                                     col * S + sg * 128:
                                     col * S + sg * 128 + 128],
                            rhs=qT_sb[half * 64:half * 64 + 64,
                                      col * TS:(col + 1) * TS],
                            start=(dve or em), stop=True)
                    pt = streamp.tile([128, 2 * TS], fp16, tag="P", bufs=8,
                                      name=f"pt{col}_{sg}")
                    if dve:
                        for half in range(2):
                            nc.vector.scalar_tensor_tensor(
                                out=pt[:, half * TS:(half + 1) * TS]
                                .bitcast(i16),
                                in0=qkt[:, half * TS:(half + 1) * TS],
                                scalar=float(A16),
                                in1=bmask_sb[:, mi * TS:(mi + 1) * TS],
                                op0=ALU.mult, op1=ALU.add)
                    elif em:
                        es = streamp.tile([128, 2 * TS], fp16, tag="E",
                                          bufs=4, name=f"es{col}_{sg}")
                        nc.scalar.activation(es[:], qkt[:], AF.Exp,
                                             scale=0.125)
                        eng = nc.gpsimd if sg in GP_SGS else nc.vector
                        for half in range(2):
                            eng.tensor_tensor(
                                out=pt[:, half * TS:(half + 1) * TS],
                                in0=es[:, half * TS:(half + 1) * TS],
                                in1=emask_sb[:, mi * TS:(mi + 1) * TS],
                                op=ALU.mult)
                    else:
                        nc.scalar.activation(pt[:], qkt[:], AF.Exp,
                                             scale=0.125)
                    pts[sg] = pt

                def emit_pv(sg, col=col, pv=pv, pts=pts):
                    pt = pts.pop(sg)
                    for half in range(2):
                        kg = (2 * col + half) * SB + sg  # v65 column order
                        nc.tensor.matmul(
                            pv[half][:],
                            lhsT=v2[:, kg * (DQ + 1):(kg + 1) * (DQ + 1)],
                            rhs=pt[:, half * TS:(half + 1) * TS],
                            start=(sg == 0), stop=(sg == SB - 1))

                for sg in range(SB):
                    emit_qk_exp(sg)
                    if sg >= PIPE_LAG:
                        emit_pv(sg - PIPE_LAG)
                    if col == 0 and sg == 8:
                        qproj(1)
                    if col == 0 and sg == 10:
                        qproj(3)
                for sg in range(SB - PIPE_LAG, SB):
                    emit_pv(sg)

                for half in range(2):
                    ob = streamp.tile([DQ + 1, TS], f32, tag="ob", bufs=4,
                                      name=f"ob{col}_{half}")
                    if half == 0:
                        nc.scalar.copy(ob[:], pv[half][:])
                        nc.sync.dma_start(out=o[col + 2 * half], in_=ob[:])
                    else:
                        nc.vector.tensor_copy(ob[:], pv[half][:])
                        nc.scalar.dma_start(out=o[col + 2 * half], in_=ob[:])
    nc.compile()
    return nc


def _get_l1():
    if "l1" not in _CACHE:
        _CACHE["l1"] = _build_l1()
    return _CACHE["l1"]


def _get_l2():
    if "l2" not in _CACHE:
        _CACHE["l2"] = _build_l2()
    return _CACHE["l2"]


def make_in_maps_l1(src, wk, wv):
    src16 = np.asarray(src).astype(FP16).reshape(B * S, D)
    wkv = np.concatenate([np.asarray(wk), np.asarray(wv)],
                         axis=1).astype(FP16)
    maps = []
    for c in CORES:
        # per-quarter SBUF image: [q][p, (j, t)] <- src[c*SR + q*512 + t,
        # j*128 + p]
        sl = src16[c * SR:(c + 1) * SR, :]                   # [2048, 1024]
        img = np.ascontiguousarray(
            sl.reshape(4, 512, 8, 128).transpose(0, 3, 2, 1)  # [q, p, j, t]
            .reshape(4, 128, 8 * 512))
        maps.append({"srcq": img, "wkv": wkv})
    return maps


def glue_l1_outputs(results):
    """Assemble kt2 / v65 from the 8 per-core kvT outputs (layout only)."""
    kvs = [np.asarray(results[c]["kvT"]) for c in CORES]
    kT_full = np.concatenate([kv[0:64] for kv in kvs], axis=1)   # [64, B*S]
    kt2 = np.ascontiguousarray(
        np.concatenate([kT_full[:, :2 * S], kT_full[:, 2 * S:]], axis=0))
    v_full = np.concatenate([kv[64:128] for kv in kvs], axis=1).T  # [B*S, 64]
    v65 = np.empty((B * S, DQ + 1), dtype=FP16)
    v65[:, :DQ] = v_full
    v65[:, DQ] = np.asarray(1.0, dtype=FP16)
    v65 = v65.reshape(B, SB, 128, DQ + 1)[[0, 2, 1, 3]]  # batch-column order
    v65 = np.ascontiguousarray(
        v65.reshape(GK, 128, DQ + 1).transpose(1, 0, 2).reshape(128, -1))
    return kt2, v65


def make_in_maps_l2(kt2, v65, tgt, mask, wq, bq, bv):
    tgt = np.asarray(tgt)
    mask = np.ascontiguousarray(mask, dtype=F32)
    wq16 = np.asarray(wq).astype(FP16)
    bq = np.ascontiguousarray(bq, dtype=F32)
    maps = []
    for c in CORES:
        # SBUF image: [b][p, (half, j, t)]  (qproj rhs chunks, contiguous)
        tgc = tgt[:, c * TS:(c + 1) * TS, :].astype(FP16)       # [B, TS, D]
        tgi = np.ascontiguousarray(
            tgc.transpose(0, 2, 1)                              # [B, D, TS]
            .reshape(B, 8, 128, TS).transpose(0, 2, 1, 3)       # [B, p, j, t]
            .reshape(B, 128, 8 * TS))
        m = {"kt2": kt2, "v65": v65, "tgt": tgi, "wq": wq16, "bq": bq}
        masknT = mask[c * TS:(c + 1) * TS, :].T  # [S, TS]: [s, t]

        def img(sgs, arr):
            # [128, n*TS] SBUF image: col g*TS+t <- arr[sgs[g]*128+p, t]
            sub = np.stack([arr[g * 128:(g + 1) * 128] for g in sgs], axis=1)
            return np.ascontiguousarray(sub.reshape(128, len(sgs) * TS))

        if N_ASG:
            m["maskA"] = img(ACT_SGS, masknT.astype(FP16))
        if N_DSG:
            m["bmask"] = img(
                DVE_SGS, np.rint(masknT * A16 + B16C).astype(np.int16))
        if N_GSG:
            m["emask"] = img(EM_SGS, np.exp(masknT * 0.125).astype(FP16))
        maps.append(m)
    return maps


def kernel(src, tgt, mask, wq, bq, wk, bk, wv, bv):
    from concourse.bass_utils import run_bass_kernel_spmd

    res1 = run_bass_kernel_spmd(_get_l1(), make_in_maps_l1(src, wk, wv),
                                core_ids=CORES)
    kt2, v65 = glue_l1_outputs(res1.results)
    res2 = run_bass_kernel_spmd(
        _get_l2(), make_in_maps_l2(kt2, v65, tgt, mask, wq, bq, bv),
        core_ids=CORES)
    bv = np.ascontiguousarray(bv, dtype=F32)
    out = np.empty((B, S, DQ), dtype=F32)
    for c in CORES:
        oc = np.asarray(res2.results[c]["o"])          # [B, 65, TS] f32
        att = oc[:, :DQ, :] / oc[:, DQ:DQ + 1, :]      # softmax division
        out[:, c * TS:(c + 1) * TS, :] = \
            att.transpose(0, 2, 1) + bv[None, None, :]
    return out


# revision 37
# speedup vs baseline: 1.0118x; 1.0118x over previous
"""Trainium2 Bass kernel for single-head cross-attention with additive mask.

Computation (matches the reference):
    q = tgt @ wq + bq
    k = src @ wk (+ bk dropped: softmax cancels a per-row constant exactly)
    v = src @ wv (bv applied on host in the epilogue)
    s = (q k^T + mask) / sqrt(DQ)
    out = softmax(s) @ v + bv

Two SPMD launches on 8 cores (all matmul inputs fp16, fp32 PSUM accum):
  L1: each core projects K and V for 1/8 of the global (B*S) src rows in a
      single fused matmul (wk|wv concatenated -> kvT [128, 2048] fp16 out),
      pipelined per 512-row quarter (DMA -> 8 matmuls -> evac -> store).
  host: pure layout glue -- assembles kt2 (d-major K) and v65 (V with an
      appended ones column for the softmax denominator).
  L2: tgt sharded 8 ways; core c handles tgt rows [c*512,(c+1)*512) of every
      batch so its mask slice is read from HBM exactly once.

L2 computes scores transposed (src-block on PSUM partitions) batch-column
outer: column c processes batches {c, c+2}.  exp((qk+mask)/8) is split
across three engines to clear the scalar-engine bottleneck (ACT is 1
elem/lane/cycle, a 55us floor if it ran every exp alone):
  - ACT_SGS: mask enters PSUM ahead of QK via an identity-weight matmul,
    ACT reads (qk+mask) from PSUM and emits fp16 exp at scale=1/8;
  - GP_SGS: ACT emits exp(qk/8), the otherwise-idle GPSIMD multiplies by
    host-baked exp(mask/8) (placed early in the loop -- longest chain);
  - DVE_SGS: the vector engine computes a Schraudolph bit-trick exp in the
    fp16 bit domain: bits16 = int16(qk*A16 + bmask) where bmask (host
    int16) carries mask*A16 + (15-sigma)*1024; bitcast to fp16 IS the
    approximate exp (rel err ~3%, diluted to ~1.2e-2 end-to-end).
PV matmul emission trails QK by PIPE_LAG blocks so the tensor engine never
idles waiting for an exp (keeps HAM un-throttled; the PE is the bottleneck
at its 16-bit streaming roofline).  All DMA rides the two HW-DGE engines
(sync/scalar) as host-prearranged contiguous SBUF images, issued in
need-order so the q projection's tgt slices land first.  PV accumulates
fp32 in PSUM with a 65th "ones" output row; the final division by the
softmax denominator (+bv) runs on the host.
"""
import numpy as np

B, S, D, DQ = 4, 4096, 1024, 64
NCORES = 8
TS = S // NCORES            # 512 tgt rows per core
SR = (B * S) // NCORES      # 2048 global src rows per core (L1)
SB = S // 128               # 32 src blocks per batch
GK = B * SB                 # 128 global src blocks
CORES = list(range(NCORES))
F32 = np.float32
FP16 = np.float16
PIPE_LAG = 5

# --- per-src-block exp-path assignment ---
# S: DVE Schraudolph bit-trick (mask folded into the int16 affine bias)
# G: exact ACT exp, then GPSIMD multiply by host-baked exp(mask/8)
# V: exact ACT exp, then DVE multiply by exp(mask/8)
# M: mask into PSUM via identity matmul, then exact ACT exp
N_DVE_SG = 14
SIGMA = 0.035
A16 = (2.0 ** 10) * np.log2(np.e) / 8.0
B16C = (2.0 ** 10) * (15.0 - SIGMA)
# GP blocks early (their exp->gpsimd-multiply chain is the longest, so it
# must not gate the end of a column); mask-path blocks last (shortest
# non-PE chain); Schraudolph fills the rest.
GP_SGS = (0, 3, 6, 9, 12, 15, 18, 21)
ACT_SGS = (5, 13, 17, 25, 26, 27, 28, 29, 30, 31)
V2_SGS = ()
DVE_SGS = tuple(g for g in range(SB)
                if g not in GP_SGS and g not in ACT_SGS)     # 14 blocks
EM_SGS = tuple(sorted(GP_SGS + V2_SGS))                      # share emask
N_ASG, N_DSG, N_GSG = len(ACT_SGS), len(DVE_SGS), len(EM_SGS)
# position of each sg within its path's packed mask array
_MASK_IDX = {g: i for i, g in enumerate(ACT_SGS)}
_MASK_IDX.update({g: i for i, g in enumerate(DVE_SGS)})
_MASK_IDX.update({g: i for i, g in enumerate(EM_SGS)})

_CACHE = {}


def _build_l1():
    import concourse.mybir as mybir
    import concourse.tile as tile
    from concourse import bacc

    f32 = mybir.dt.float32
    fp16 = mybir.dt.float16

    nc = bacc.Bacc("TRN2", target_bir_lowering=False, debug=False,
                   num_devices=NCORES)
    # host-prearranged SBUF image per src-row quarter: [q][p, (j, t)]
    srcq = nc.dram_tensor("srcq", [4, 128, 8 * 512], fp16,
                          kind="ExternalInput")
    # host-prearranged SBUF image: [p, (j, m)] <- wkv[j*128 + p, m]
    wkv = nc.dram_tensor("wkv", [128, 8 * 128], fp16, kind="ExternalInput")
    kvT = nc.dram_tensor("kvT", [128, SR], fp16, kind="ExternalOutput")

    with tile.TileContext(nc) as tc:
        with (
            tc.tile_pool(name="const", bufs=1) as constp,
            tc.tile_pool(name="big", bufs=1) as bigp,
            tc.tile_pool(name="stream", bufs=2) as streamp,
            tc.tile_pool(name="pp", bufs=1, space="PSUM") as pp,
        ):
            wkv_sb = constp.tile([128, 8 * 128], fp16)
            nc.sync.dma_start(out=wkv_sb[:], in_=wkv[:])
            sts = []
            for q in range(4):
                st = streamp.tile([128, 8 * 512], fp16, tag="xs", bufs=4,
                                  name=f"st{q}")
                eng = nc.sync if q % 2 == 0 else nc.scalar
                eng.dma_start(out=st[:], in_=srcq[q])
                sts.append(st)
            kv_sb = bigp.tile([128, SR], fp16)
            # PE warm-up during the src DMA wait (see L2 comment)
            warm = pp.tile([128, 512], f32, tag="kv", bufs=2, name="warm")
            for _ in range(10):
                nc.tensor.matmul(warm[:], lhsT=wkv_sb[:, 0:128],
                                 rhs=wkv_sb[:, 0:512], start=True, stop=True)
            # per-quarter pipeline: 8 matmuls -> evac -> store, while the
            # next quarter's src image is still in flight
            for q in range(4):
                kv_ps = pp.tile([128, 512], f32, tag="kv", bufs=2,
                                name=f"kv_ps{q}")
                for j in range(8):
                    nc.tensor.matmul(
                        kv_ps[:],
                        lhsT=wkv_sb[:, j * 128:(j + 1) * 128],
                        rhs=sts[q][:, j * 512:(j + 1) * 512],
                        start=(j == 0), stop=(j == 7))
                if q % 2 == 0:
                    nc.scalar.copy(kv_sb[:, q * 512:(q + 1) * 512],
                                   kv_ps[:])
                else:
                    nc.vector.tensor_copy(kv_sb[:, q * 512:(q + 1) * 512],
                                          kv_ps[:])
                eng = nc.sync if q % 2 == 0 else nc.scalar
                eng.dma_start(out=kvT[:, q * 512:(q + 1) * 512],
                              in_=kv_sb[:, q * 512:(q + 1) * 512])
    nc.compile()
    return nc


def _build_l2():
    import concourse.mybir as mybir
    import concourse.tile as tile
    from concourse import bacc
    from concourse.masks import make_identity

    f32 = mybir.dt.float32
    fp16 = mybir.dt.float16
    i16 = mybir.dt.int16
    AF = mybir.ActivationFunctionType
    ALU = mybir.AluOpType

    nc = bacc.Bacc("TRN2", target_bir_lowering=False, debug=False,
                   num_devices=NCORES)
    # kt2: partitions 0-63 = kT of batches 0|1; 64-127 = batches 2|3
    kt2d = nc.dram_tensor("kt2", [128, 2 * S], fp16, kind="ExternalInput")
    # v65, batch-column order (b0, b2, b1, b3): row p, col (bb, kg', c):
    # element = v[b(bb), kg'*128 + p, c] | ones
    v65d = nc.dram_tensor("v65", [128, GK * (DQ + 1)], fp16,
                          kind="ExternalInput")
    # host-prearranged SBUF image: tgt[b][p, (half, j, t)] (contiguous rows)
    tgtd = nc.dram_tensor("tgt", [B, 128, 8 * TS], fp16,
                          kind="ExternalInput")
    # packed SBUF-image mask rows for the ACT-path src blocks (fp16)
    if N_ASG:
        maskA = nc.dram_tensor("maskA", [128, N_ASG * TS], fp16,
                               kind="ExternalInput")
    # packed Schraudolph bias image for the DVE-path src blocks (int16)
    if N_DSG:
        bmaskd = nc.dram_tensor("bmask", [128, N_DSG * TS], i16,
                                kind="ExternalInput")
    # packed exp(mask/8) image for the GPSIMD-multiply src blocks (fp16)
    if N_GSG:
        emaskd = nc.dram_tensor("emask", [128, N_GSG * TS], fp16,
                                kind="ExternalInput")
    # host-prearranged SBUF image: [p, (j, m)] <- wq[j*128 + p, m]
    wq = nc.dram_tensor("wq", [128, 8 * DQ], fp16, kind="ExternalInput")
    bq = nc.dram_tensor("bq", [128, 1], f32, kind="ExternalInput")
    # out rows 0-63: (attn @ v)^T numerator; row 64: softmax denominator
    o = nc.dram_tensor("o", [B, DQ + 1, TS], f32, kind="ExternalOutput")

    with tile.TileContext(nc) as tc:
        with (
            tc.tile_pool(name="const", bufs=1) as constp,
            tc.tile_pool(name="big", bufs=1) as bigp,
            tc.tile_pool(name="stream", bufs=2) as streamp,
            tc.tile_pool(name="pp", bufs=1, space="PSUM") as pp,
        ):
            # ---- constants + all input DMA, issued in need-order
            wq_sb = constp.tile([128, 8 * DQ], fp16)
            bq_sb = constp.tile([128, 1], f32)
            ident = constp.tile([128, 128], fp16)
            make_identity(nc, ident[:])

            kT2 = bigp.tile([128, 2 * S], fp16)
            v2 = bigp.tile([128, GK * (DQ + 1)], fp16)
            VQ = 32 * (DQ + 1)
            if N_ASG:
                maskA_sb = bigp.tile([128, N_ASG * TS], fp16)
            if N_DSG:
                bmask_sb = bigp.tile([128, N_DSG * TS], i16)
            if N_GSG:
                emask_sb = bigp.tile([128, N_GSG * TS], fp16)
            tgs = {}
            for b in (0, 2, 1, 3):
                tgs[b] = streamp.tile([128, 8 * TS], fp16, tag="tg", bufs=4,
                                      name=f"tg{b}")

            def load_tg(b, eng):
                eng.dma_start(out=tgs[b][:], in_=tgtd[b])

            def load_kt2(lo, hi, eng):
                eng.dma_start(out=kT2[:, lo:hi], in_=kt2d[:, lo:hi])

            def load_v2(i0, eng):
                eng.dma_start(out=v2[:, i0 * VQ:(i0 + 2) * VQ],
                              in_=v65d[:, i0 * VQ:(i0 + 2) * VQ])

            def load_img(sb_tile, dram, lo, hi, n, eng):
                hi = min(hi, n)
                if n and lo < hi:
                    eng.dma_start(out=sb_tile[:, lo * TS:hi * TS],
                                  in_=dram[:, lo * TS:hi * TS])

            # need-ordered issue, split across the two HW-DGE sequencers;
            # tiny first chunks so sg 0 unblocks as early as possible
            nc.scalar.dma_start(out=wq_sb[:], in_=wq[:])
            nc.sync.dma_start(out=bq_sb[:], in_=bq[:])
            load_tg(0, nc.sync)
            load_tg(2, nc.scalar)
            load_kt2(0, 512, nc.sync)       # column 0, sg 0-3
            if N_DSG:
                load_img(bmask_sb, bmaskd, 0, 3, N_DSG, nc.scalar)
            if N_GSG:
                load_img(emask_sb, emaskd, 0, 3, N_GSG, nc.scalar)
            if N_ASG:
                load_img(maskA_sb, maskA, 0, N_ASG, N_ASG, nc.scalar)
            load_kt2(512, 2048, nc.scalar)  # column 0, sg 4-15
            if N_DSG:
                load_img(bmask_sb, bmaskd, 3, 8, N_DSG, nc.sync)
            if N_GSG:
                load_img(emask_sb, emaskd, 3, 8, N_GSG, nc.scalar)
            load_v2(0, nc.sync)             # column-0 batches (b0, b2)
            if N_DSG:
                load_img(bmask_sb, bmaskd, 8, N_DSG, N_DSG, nc.sync)
            if N_GSG:
                load_img(emask_sb, emaskd, 8, N_GSG, N_GSG, nc.scalar)
            load_kt2(2048, 4096, nc.scalar)  # column 0, sg 16-31
            load_tg(1, nc.sync)
            load_tg(3, nc.sync)
            load_kt2(4096, 8192, nc.scalar)  # column 1
            load_v2(2, nc.sync)             # column-1 batches (b1, b3)

            # PE warm-up: the HAM clock gate needs ~3.4us of sustained matmul
            # activity to lift the 1.2->2.4 GHz throttle.  The PE is idle
            # while the first tgt/kt2 slices stream in, so burn that window
            # on dummy matmuls over the (tiny, already-landed) wq tile; the
            # q projection and the first score blocks then run at full clock.
            warm = pp.tile([128, TS], f32, tag="qk", bufs=3, name="warm")
            for _ in range(16):
                nc.tensor.matmul(warm[:], lhsT=wq_sb[:, 0:128],
                                 rhs=wq_sb[:, 0:512], start=True, stop=True)

            # ---- q projection (column order; b1/b3 emitted mid-loop below)
            qT_sb = bigp.tile([128, 2 * TS], fp16)

            def qproj(b):
                pb, colb = (b // 2) * 64, (b % 2) * TS
                q_ps = pp.tile([128, TS], f32, tag="qk", bufs=3,
                               name=f"q_ps{b}")
                for j in range(8):
                    nc.tensor.matmul(
                        q_ps[pb:pb + 64, :],
                        lhsT=wq_sb[:, j * DQ:(j + 1) * DQ],
                        rhs=tgs[b][:, j * TS:(j + 1) * TS],
                        start=(j == 0), stop=(j == 7))
                nc.scalar.activation(
                    qT_sb[pb:pb + 64, colb:colb + TS], q_ps[pb:pb + 64, :],
                    AF.Identity, bias=bq_sb[pb:pb + 64, :])

            qproj(0)
            qproj(2)

            # ---- attention main loop: batch-column outer; column c handles
            # batches {c, c+2} on disjoint 64-row PE tiles.  PV emission
            # trails by PIPE_LAG so the PE never waits on an exp.
            for col in range(2):
                pv = [pp.tile([DQ + 1, TS], f32, tag=f"pv{h}",
                              name=f"pv{col}_{h}") for h in range(2)]
                pts = {}

                def emit_qk_exp(sg, col=col, pts=pts):
                    dve = sg in DVE_SGS
                    em = sg in EM_SGS
                    mi = _MASK_IDX[sg]
                    qkt = pp.tile([128, 2 * TS], f32, tag="qk", bufs=3,
                                  name=f"qkt{col}_{sg}")
                    if not (dve or em):
                        for half in range(2):
                            nc.tensor.matmul(
                                qkt[:, half * TS:(half + 1) * TS],
                                lhsT=ident[:],
                                rhs=maskA_sb[:, mi * TS:(mi + 1) * TS],
                                start=True, stop=False)
                    for half in range(2):
                        nc.tensor.matmul(
                            qkt[:, half * TS:(half + 1) * TS],
                            lhsT=kT2[half * 64:half * 64 + 64,
                                     col * S + sg * 128:
                                     col * S + sg * 128 + 128],
                            rhs=qT_sb[half * 64:half * 64 + 64,
                                      col * TS:(col + 1) * TS],
                            start=(dve or em), stop=True)
                    pt = streamp.tile([128, 2 * TS], fp16, tag="P", bufs=8,
                                      name=f"pt{col}_{sg}")
                    if dve:
                        for half in range(2):
                            nc.vector.scalar_tensor_tensor(
                                out=pt[:, half * TS:(half + 1) * TS]
                                .bitcast(i16),
                                in0=qkt[:, half * TS:(half + 1) * TS],
                                scalar=float(A16),
                                in1=bmask_sb[:, mi * TS:(mi + 1) * TS],
                                op0=ALU.mult, op1=ALU.add)
                    elif em:
                        es = streamp.tile([128, 2 * TS], fp16, tag="E",
                                          bufs=4, name=f"es{col}_{sg}")
                        nc.scalar.activation(es[:], qkt[:], AF.Exp,
                                             scale=0.125)
                        eng = nc.gpsimd if sg in GP_SGS else nc.vector
                        for half in range(2):
                            eng.tensor_tensor(
                                out=pt[:, half * TS:(half + 1) * TS],
                                in0=es[:, half * TS:(half + 1) * TS],
                                in1=emask_sb[:, mi * TS:(mi + 1) * TS],
                                op=ALU.mult)
                    else:
                        nc.scalar.activation(pt[:], qkt[:], AF.Exp,
                                             scale=0.125)
                    pts[sg] = pt

                def emit_pv(sg, col=col, pv=pv, pts=pts):
                    pt = pts.pop(sg)
                    for half in range(2):
                        kg = (2 * col + half) * SB + sg  # v65 column order
                        nc.tensor.matmul(
                            pv[half][:],
                            lhsT=v2[:, kg * (DQ + 1):(kg + 1) * (DQ + 1)],
                            rhs=pt[:, half * TS:(half + 1) * TS],
                            start=(sg == 0), stop=(sg == SB - 1))

                for sg in range(SB):
                    emit_qk_exp(sg)
                    if sg >= PIPE_LAG:
                        emit_pv(sg - PIPE_LAG)
                    if col == 0 and sg == 8:
                        qproj(1)
                    if col == 0 and sg == 10:
                        qproj(3)
                for sg in range(SB - PIPE_LAG, SB):
                    emit_pv(sg)

                for half in range(2):
                    ob = streamp.tile([DQ + 1, TS], f32, tag="ob", bufs=4,
                                      name=f"ob{col}_{half}")
                    if half == 0:
                        nc.scalar.copy(ob[:], pv[half][:])
                        nc.sync.dma_start(out=o[col + 2 * half], in_=ob[:])
                    else:
                        nc.vector.tensor_copy(ob[:], pv[half][:])
                        nc.scalar.dma_start(out=o[col + 2 * half], in_=ob[:])
    nc.compile()
    return nc


def _get_l1():
    if "l1" not in _CACHE:
        _CACHE["l1"] = _build_l1()
    return _CACHE["l1"]


def _get_l2():
    if "l2" not in _CACHE:
        _CACHE["l2"] = _build_l2()
    return _CACHE["l2"]


def make_in_maps_l1(src, wk, wv):
    src16 = np.asarray(src).astype(FP16).reshape(B * S, D)
    wkv = np.concatenate([np.asarray(wk), np.asarray(wv)],
                         axis=1).astype(FP16)
    wkv = np.ascontiguousarray(
        wkv.reshape(8, 128, 128).transpose(1, 0, 2).reshape(128, 8 * 128))
    maps = []
    for c in CORES:
        # per-quarter SBUF image: [q][p, (j, t)] <- src[c*SR + q*512 + t,
        # j*128 + p]
        sl = src16[c * SR:(c + 1) * SR, :]                   # [2048, 1024]
        img = np.ascontiguousarray(
            sl.reshape(4, 512, 8, 128).transpose(0, 3, 2, 1)  # [q, p, j, t]
            .reshape(4, 128, 8 * 512))
        maps.append({"srcq": img, "wkv": wkv})
    return maps


def glue_l1_outputs(results):
    """Assemble kt2 / v65 from the 8 per-core kvT outputs (layout only)."""
    kvs = [np.asarray(results[c]["kvT"]) for c in CORES]
    kT_full = np.concatenate([kv[0:64] for kv in kvs], axis=1)   # [64, B*S]
    kt2 = np.ascontiguousarray(
        np.concatenate([kT_full[:, :2 * S], kT_full[:, 2 * S:]], axis=0))
    v_full = np.concatenate([kv[64:128] for kv in kvs], axis=1).T  # [B*S, 64]
    v65 = np.empty((B * S, DQ + 1), dtype=FP16)
    v65[:, :DQ] = v_full
    v65[:, DQ] = np.asarray(1.0, dtype=FP16)
    v65 = v65.reshape(B, SB, 128, DQ + 1)[[0, 2, 1, 3]]  # batch-column order
    v65 = np.ascontiguousarray(
        v65.reshape(GK, 128, DQ + 1).transpose(1, 0, 2).reshape(128, -1))
    return kt2, v65


def make_in_maps_l2(kt2, v65, tgt, mask, wq, bq, bv):
    tgt = np.asarray(tgt)
    mask = np.ascontiguousarray(mask, dtype=F32)
    wq16 = np.asarray(wq).astype(FP16)
    wq16 = np.ascontiguousarray(
        wq16.reshape(8, 128, DQ).transpose(1, 0, 2).reshape(128, 8 * DQ))
    bq = np.asarray(bq, dtype=F32)
    bq = np.ascontiguousarray(np.concatenate([bq, bq]).reshape(128, 1))
    maps = []
    for c in CORES:
        # SBUF image: [b][p, (half, j, t)]  (qproj rhs chunks, contiguous)
        tgc = tgt[:, c * TS:(c + 1) * TS, :].astype(FP16)       # [B, TS, D]
        tgi = np.ascontiguousarray(
            tgc.transpose(0, 2, 1)                              # [B, D, TS]
            .reshape(B, 8, 128, TS).transpose(0, 2, 1, 3)       # [B, p, j, t]
            .reshape(B, 128, 8 * TS))
        m = {"kt2": kt2, "v65": v65, "tgt": tgi, "wq": wq16, "bq": bq}
        masknT = mask[c * TS:(c + 1) * TS, :].T  # [S, TS]: [s, t]

        def img(sgs, arr):
            # [128, n*TS] SBUF image: col g*TS+t <- arr[sgs[g]*128+p, t]
            sub = np.stack([arr[g * 128:(g + 1) * 128] for g in sgs], axis=1)
            return np.ascontiguousarray(sub.reshape(128, len(sgs) * TS))

        if N_ASG:
            m["maskA"] = img(ACT_SGS, masknT.astype(FP16))
        if N_DSG:
            m["bmask"] = img(
                DVE_SGS, np.rint(masknT * A16 + B16C).astype(np.int16))
        if N_GSG:
            m["emask"] = img(EM_SGS, np.exp(masknT * 0.125).astype(FP16))
        maps.append(m)
    return maps


def kernel(src, tgt, mask, wq, bq, wk, bk, wv, bv):
    from concourse.bass_utils import run_bass_kernel_spmd

    res1 = run_bass_kernel_spmd(_get_l1(), make_in_maps_l1(src, wk, wv),
                                core_ids=CORES)
    kt2, v65 = glue_l1_outputs(res1.results)
    res2 = run_bass_kernel_spmd(
        _get_l2(), make_in_maps_l2(kt2, v65, tgt, mask, wq, bq, bv),
        core_ids=CORES)
    bv = np.ascontiguousarray(bv, dtype=F32)
    out = np.empty((B, S, DQ), dtype=F32)
    for c in CORES:
        oc = np.asarray(res2.results[c]["o"])          # [B, 65, TS] f32
        att = oc[:, :DQ, :] / oc[:, DQ:DQ + 1, :]      # softmax division
        out[:, c * TS:(c + 1) * TS, :] = \
            att.transpose(0, 2, 1) + bv[None, None, :]
    return out


# revision 38
# speedup vs baseline: 1.0253x; 1.0133x over previous
"""Trainium2 Bass kernel for single-head cross-attention with additive mask.

Computation (matches the reference):
    q = tgt @ wq + bq
    k = src @ wk (+ bk dropped: softmax cancels a per-row constant exactly)
    v = src @ wv (bv applied on host in the epilogue)
    s = (q k^T + mask) / sqrt(DQ)
    out = softmax(s) @ v + bv

Two SPMD launches on 8 cores (all matmul inputs fp16, fp32 PSUM accum):
  L1: each core projects K and V for 1/8 of the global (B*S) src rows in a
      single fused matmul (wk|wv concatenated -> kvT [128, 2048] fp16 out),
      pipelined per 512-row quarter (DMA -> 8 matmuls -> evac -> store).
  host: pure layout glue -- assembles kt2 (d-major K) and v65 (V with an
      appended ones column for the softmax denominator).
  L2: tgt sharded 8 ways; core c handles tgt rows [c*512,(c+1)*512) of every
      batch so its mask slice is read from HBM exactly once.

L2 computes scores transposed (src-block on PSUM partitions) batch-column
outer: column c processes batches {c, c+2}.  exp((qk+mask)/8) is split
across three engines to clear the scalar-engine bottleneck (ACT is 1
elem/lane/cycle, a 55us floor if it ran every exp alone):
  - ACT_SGS: mask enters PSUM ahead of QK via an identity-weight matmul,
    ACT reads (qk+mask) from PSUM and emits fp16 exp at scale=1/8;
  - GP_SGS: ACT emits exp(qk/8), the otherwise-idle GPSIMD multiplies by
    host-baked exp(mask/8) (placed early in the loop -- longest chain);
  - DVE_SGS: the vector engine computes a Schraudolph bit-trick exp in the
    fp16 bit domain: bits16 = int16(qk*A16 + bmask) where bmask (host
    int16) carries mask*A16 + (15-sigma)*1024; bitcast to fp16 IS the
    approximate exp (rel err ~3%, diluted to ~1.2e-2 end-to-end).
PV matmul emission trails QK by PIPE_LAG blocks so the tensor engine never
idles waiting for an exp (keeps HAM un-throttled; the PE is the bottleneck
at its 16-bit streaming roofline).  All DMA rides the two HW-DGE engines
(sync/scalar) as host-prearranged contiguous SBUF images, issued in
need-order so the q projection's tgt slices land first.  PV accumulates
fp32 in PSUM with a 65th "ones" output row; the final division by the
softmax denominator (+bv) runs on the host.
"""
import numpy as np

B, S, D, DQ = 4, 4096, 1024, 64
NCORES = 8
TS = S // NCORES            # 512 tgt rows per core
SR = (B * S) // NCORES      # 2048 global src rows per core (L1)
SB = S // 128               # 32 src blocks per batch
GK = B * SB                 # 128 global src blocks
CORES = list(range(NCORES))
F32 = np.float32
FP16 = np.float16
PIPE_LAG = 5

# --- per-src-block exp-path assignment ---
# S: DVE Schraudolph bit-trick (mask folded into the int16 affine bias)
# G: exact ACT exp, then GPSIMD multiply by host-baked exp(mask/8)
# V: exact ACT exp, then DVE multiply by exp(mask/8)
# M: mask into PSUM via identity matmul, then exact ACT exp
N_DVE_SG = 14
SIGMA = 0.035
A16 = (2.0 ** 10) * np.log2(np.e) / 8.0
B16C = (2.0 ** 10) * (15.0 - SIGMA)
# GP blocks early (their exp->gpsimd-multiply chain is the longest, so it
# must not gate the end of a column); mask-path blocks last (shortest
# non-PE chain); Schraudolph fills the rest.
GP_SGS = (0, 3, 6, 9, 12, 15, 18, 21)
ACT_SGS = (17, 25, 26, 27, 28, 29, 30, 31)
V2_SGS = ()
DVE_SGS = tuple(g for g in range(SB)
                if g not in GP_SGS and g not in ACT_SGS)     # 16 blocks
EM_SGS = tuple(sorted(GP_SGS + V2_SGS))                      # share emask
N_ASG, N_DSG, N_GSG = len(ACT_SGS), len(DVE_SGS), len(EM_SGS)
# position of each sg within its path's packed mask array
_MASK_IDX = {g: i for i, g in enumerate(ACT_SGS)}
_MASK_IDX.update({g: i for i, g in enumerate(DVE_SGS)})
_MASK_IDX.update({g: i for i, g in enumerate(EM_SGS)})

_CACHE = {}


def _build_l1():
    import concourse.mybir as mybir
    import concourse.tile as tile
    from concourse import bacc

    f32 = mybir.dt.float32
    fp16 = mybir.dt.float16

    nc = bacc.Bacc("TRN2", target_bir_lowering=False, debug=False,
                   num_devices=NCORES)
    # host-prearranged SBUF image per src-row quarter: [q][p, (j, t)]
    srcq = nc.dram_tensor("srcq", [4, 128, 8 * 512], fp16,
                          kind="ExternalInput")
    # host-prearranged SBUF image: [p, (j, m)] <- wkv[j*128 + p, m]
    wkv = nc.dram_tensor("wkv", [128, 8 * 128], fp16, kind="ExternalInput")
    kvT = nc.dram_tensor("kvT", [128, SR], fp16, kind="ExternalOutput")

    with tile.TileContext(nc) as tc:
        with (
            tc.tile_pool(name="const", bufs=1) as constp,
            tc.tile_pool(name="big", bufs=1) as bigp,
            tc.tile_pool(name="stream", bufs=2) as streamp,
            tc.tile_pool(name="pp", bufs=1, space="PSUM") as pp,
        ):
            wkv_sb = constp.tile([128, 8 * 128], fp16)
            nc.sync.dma_start(out=wkv_sb[:], in_=wkv[:])
            sts = []
            for q in range(4):
                st = streamp.tile([128, 8 * 512], fp16, tag="xs", bufs=4,
                                  name=f"st{q}")
                eng = nc.sync if q % 2 == 0 else nc.scalar
                eng.dma_start(out=st[:], in_=srcq[q])
                sts.append(st)
            kv_sb = bigp.tile([128, SR], fp16)
            # PE warm-up during the src DMA wait (see L2 comment)
            warm = pp.tile([128, 512], f32, tag="kv", bufs=2, name="warm")
            for _ in range(8):
                nc.tensor.matmul(warm[:], lhsT=wkv_sb[:, 0:128],
                                 rhs=wkv_sb[:, 0:512], start=True, stop=True)
            # per-quarter pipeline: 8 matmuls -> evac -> store, while the
            # next quarter's src image is still in flight
            for q in range(4):
                kv_ps = pp.tile([128, 512], f32, tag="kv", bufs=2,
                                name=f"kv_ps{q}")
                for j in range(8):
                    nc.tensor.matmul(
                        kv_ps[:],
                        lhsT=wkv_sb[:, j * 128:(j + 1) * 128],
                        rhs=sts[q][:, j * 512:(j + 1) * 512],
                        start=(j == 0), stop=(j == 7))
                if q % 2 == 0:
                    nc.scalar.copy(kv_sb[:, q * 512:(q + 1) * 512],
                                   kv_ps[:])
                else:
                    nc.vector.tensor_copy(kv_sb[:, q * 512:(q + 1) * 512],
                                          kv_ps[:])
                eng = nc.sync if q % 2 == 0 else nc.scalar
                eng.dma_start(out=kvT[:, q * 512:(q + 1) * 512],
                              in_=kv_sb[:, q * 512:(q + 1) * 512])
    nc.compile()
    return nc


def _build_l2():
    import concourse.mybir as mybir
    import concourse.tile as tile
    from concourse import bacc
    from concourse.masks import make_identity

    f32 = mybir.dt.float32
    fp16 = mybir.dt.float16
    i16 = mybir.dt.int16
    AF = mybir.ActivationFunctionType
    ALU = mybir.AluOpType

    nc = bacc.Bacc("TRN2", target_bir_lowering=False, debug=False,
                   num_devices=NCORES)
    # kt2: partitions 0-63 = kT of batches 0|1; 64-127 = batches 2|3
    kt2d = nc.dram_tensor("kt2", [128, 2 * S], fp16, kind="ExternalInput")
    # v65, batch-column order (b0, b2, b1, b3): row p, col (bb, kg', c):
    # element = v[b(bb), kg'*128 + p, c] | ones
    v65d = nc.dram_tensor("v65", [128, GK * (DQ + 1)], fp16,
                          kind="ExternalInput")
    # host-prearranged SBUF image: tgt[b][p, (half, j, t)] (contiguous rows)
    tgtd = nc.dram_tensor("tgt", [B, 128, 8 * TS], fp16,
                          kind="ExternalInput")
    # packed SBUF-image mask rows for the ACT-path src blocks (fp16)
    if N_ASG:
        maskA = nc.dram_tensor("maskA", [128, N_ASG * TS], fp16,
                               kind="ExternalInput")
    # packed Schraudolph bias image for the DVE-path src blocks (int16)
    if N_DSG:
        bmaskd = nc.dram_tensor("bmask", [128, N_DSG * TS], i16,
                                kind="ExternalInput")
    # packed exp(mask/8) image for the GPSIMD-multiply src blocks (fp16)
    if N_GSG:
        emaskd = nc.dram_tensor("emask", [128, N_GSG * TS], fp16,
                                kind="ExternalInput")
    # host-prearranged SBUF image: [p, (j, m)] <- wq[j*128 + p, m]
    wq = nc.dram_tensor("wq", [128, 8 * DQ], fp16, kind="ExternalInput")
    bq = nc.dram_tensor("bq", [128, 1], f32, kind="ExternalInput")
    # out rows 0-63: (attn @ v)^T numerator; row 64: softmax denominator
    o = nc.dram_tensor("o", [B, DQ + 1, TS], f32, kind="ExternalOutput")

    with tile.TileContext(nc) as tc:
        with (
            tc.tile_pool(name="const", bufs=1) as constp,
            tc.tile_pool(name="big", bufs=1) as bigp,
            tc.tile_pool(name="stream", bufs=2) as streamp,
            tc.tile_pool(name="pp", bufs=1, space="PSUM") as pp,
        ):
            # ---- constants + all input DMA, issued in need-order
            wq_sb = constp.tile([128, 8 * DQ], fp16)
            bq_sb = constp.tile([128, 1], f32)
            ident = constp.tile([128, 128], fp16)
            make_identity(nc, ident[:])

            kT2 = bigp.tile([128, 2 * S], fp16)
            v2 = bigp.tile([128, GK * (DQ + 1)], fp16)
            VQ = 32 * (DQ + 1)
            if N_ASG:
                maskA_sb = bigp.tile([128, N_ASG * TS], fp16)
            if N_DSG:
                bmask_sb = bigp.tile([128, N_DSG * TS], i16)
            if N_GSG:
                emask_sb = bigp.tile([128, N_GSG * TS], fp16)
            tgs = {}
            for b in (0, 2, 1, 3):
                tgs[b] = streamp.tile([128, 8 * TS], fp16, tag="tg", bufs=4,
                                      name=f"tg{b}")

            def load_tg(b, eng):
                eng.dma_start(out=tgs[b][:], in_=tgtd[b])

            def load_kt2(lo, hi, eng):
                eng.dma_start(out=kT2[:, lo:hi], in_=kt2d[:, lo:hi])

            def load_v2(i0, eng):
                eng.dma_start(out=v2[:, i0 * VQ:(i0 + 2) * VQ],
                              in_=v65d[:, i0 * VQ:(i0 + 2) * VQ])

            def load_img(sb_tile, dram, lo, hi, n, eng):
                hi = min(hi, n)
                if n and lo < hi:
                    eng.dma_start(out=sb_tile[:, lo * TS:hi * TS],
                                  in_=dram[:, lo * TS:hi * TS])

            # need-ordered issue, split across the two HW-DGE sequencers;
            # tiny first chunks so sg 0 unblocks as early as possible
            nc.scalar.dma_start(out=wq_sb[:], in_=wq[:])
            nc.sync.dma_start(out=bq_sb[:], in_=bq[:])
            load_tg(0, nc.sync)
            load_tg(2, nc.scalar)
            load_kt2(0, 512, nc.sync)       # column 0, sg 0-3
            if N_DSG:
                load_img(bmask_sb, bmaskd, 0, 3, N_DSG, nc.scalar)
            if N_GSG:
                load_img(emask_sb, emaskd, 0, 3, N_GSG, nc.scalar)
            if N_ASG:
                load_img(maskA_sb, maskA, 0, N_ASG, N_ASG, nc.scalar)
            load_kt2(512, 2048, nc.scalar)  # column 0, sg 4-15
            if N_DSG:
                load_img(bmask_sb, bmaskd, 3, 8, N_DSG, nc.sync)
            if N_GSG:
                load_img(emask_sb, emaskd, 3, 8, N_GSG, nc.scalar)
            load_v2(0, nc.sync)             # column-0 batches (b0, b2)
            if N_DSG:
                load_img(bmask_sb, bmaskd, 8, N_DSG, N_DSG, nc.sync)
            if N_GSG:
                load_img(emask_sb, emaskd, 8, N_GSG, N_GSG, nc.scalar)
            load_kt2(2048, 4096, nc.scalar)  # column 0, sg 16-31
            load_tg(1, nc.sync)
            load_tg(3, nc.sync)
            load_kt2(4096, 8192, nc.scalar)  # column 1
            load_v2(2, nc.sync)             # column-1 batches (b1, b3)

            # PE warm-up: the HAM clock gate needs ~3.4us of sustained matmul
            # activity to lift the 1.2->2.4 GHz throttle.  The PE is idle
            # while the first tgt/kt2 slices stream in, so burn that window
            # on dummy matmuls over the (tiny, already-landed) wq tile; the
            # q projection and the first score blocks then run at full clock.
            warm = pp.tile([128, TS], f32, tag="qk", bufs=3, name="warm")
            for _ in range(10):
                nc.tensor.matmul(warm[:], lhsT=wq_sb[:, 0:128],
                                 rhs=wq_sb[:, 0:512], start=True, stop=True)

            # ---- q projection (column order; b1/b3 emitted mid-loop below)
            qT_sb = bigp.tile([128, 2 * TS], fp16)

            def qproj(b):
                pb, colb = (b // 2) * 64, (b % 2) * TS
                q_ps = pp.tile([128, TS], f32, tag="qk", bufs=3,
                               name=f"q_ps{b}")
                for j in range(8):
                    nc.tensor.matmul(
                        q_ps[pb:pb + 64, :],
                        lhsT=wq_sb[:, j * DQ:(j + 1) * DQ],
                        rhs=tgs[b][:, j * TS:(j + 1) * TS],
                        start=(j == 0), stop=(j == 7))
                nc.scalar.activation(
                    qT_sb[pb:pb + 64, colb:colb + TS], q_ps[pb:pb + 64, :],
                    AF.Identity, bias=bq_sb[pb:pb + 64, :])

            qproj(0)
            qproj(2)

            # ---- attention main loop: batch-column outer; column c handles
            # batches {c, c+2} on disjoint 64-row PE tiles.  PV emission
            # trails by PIPE_LAG so the PE never waits on an exp.
            for col in range(2):
                pv = [pp.tile([DQ + 1, TS], f32, tag=f"pv{h}",
                              name=f"pv{col}_{h}") for h in range(2)]
                pts = {}

                def emit_qk_exp(sg, col=col, pts=pts):
                    dve = sg in DVE_SGS
                    em = sg in EM_SGS
                    mi = _MASK_IDX[sg]
                    qkt = pp.tile([128, 2 * TS], f32, tag="qk", bufs=3,
                                  name=f"qkt{col}_{sg}")
                    if not (dve or em):
                        for half in range(2):
                            nc.tensor.matmul(
                                qkt[:, half * TS:(half + 1) * TS],
                                lhsT=ident[:],
                                rhs=maskA_sb[:, mi * TS:(mi + 1) * TS],
                                start=True, stop=False)
                    for half in range(2):
                        nc.tensor.matmul(
                            qkt[:, half * TS:(half + 1) * TS],
                            lhsT=kT2[half * 64:half * 64 + 64,
                                     col * S + sg * 128:
                                     col * S + sg * 128 + 128],
                            rhs=qT_sb[half * 64:half * 64 + 64,
                                      col * TS:(col + 1) * TS],
                            start=(dve or em), stop=True)
                    pt = streamp.tile([128, 2 * TS], fp16, tag="P", bufs=8,
                                      name=f"pt{col}_{sg}")
                    if dve:
                        for half in range(2):
                            nc.vector.scalar_tensor_tensor(
                                out=pt[:, half * TS:(half + 1) * TS]
                                .bitcast(i16),
                                in0=qkt[:, half * TS:(half + 1) * TS],
                                scalar=float(A16),
                                in1=bmask_sb[:, mi * TS:(mi + 1) * TS],
                                op0=ALU.mult, op1=ALU.add)
                    elif em:
                        es = streamp.tile([128, 2 * TS], fp16, tag="E",
                                          bufs=4, name=f"es{col}_{sg}")
                        nc.scalar.activation(es[:], qkt[:], AF.Exp,
                                             scale=0.125)
                        eng = nc.gpsimd if sg in GP_SGS else nc.vector
                        for half in range(2):
                            eng.tensor_tensor(
                                out=pt[:, half * TS:(half + 1) * TS],
                                in0=es[:, half * TS:(half + 1) * TS],
                                in1=emask_sb[:, mi * TS:(mi + 1) * TS],
                                op=ALU.mult)
                    else:
                        nc.scalar.activation(pt[:], qkt[:], AF.Exp,
                                             scale=0.125)
                    pts[sg] = pt

                def emit_pv(sg, col=col, pv=pv, pts=pts):
                    pt = pts.pop(sg)
                    for half in range(2):
                        kg = (2 * col + half) * SB + sg  # v65 column order
                        nc.tensor.matmul(
                            pv[half][:],
                            lhsT=v2[:, kg * (DQ + 1):(kg + 1) * (DQ + 1)],
                            rhs=pt[:, half * TS:(half + 1) * TS],
                            start=(sg == 0), stop=(sg == SB - 1))

                for sg in range(SB):
                    emit_qk_exp(sg)
                    if sg >= PIPE_LAG:
                        emit_pv(sg - PIPE_LAG)
                    if col == 0 and sg == 8:
                        qproj(1)
                    if col == 0 and sg == 10:
                        qproj(3)
                for sg in range(SB - PIPE_LAG, SB):
                    emit_pv(sg)

                for half in range(2):
                    ob = streamp.tile([DQ + 1, TS], f32, tag="ob", bufs=4,
                                      name=f"ob{col}_{half}")
                    if half == 0:
                        nc.scalar.copy(ob[:], pv[half][:])
                        nc.sync.dma_start(out=o[col + 2 * half], in_=ob[:])
                    else:
                        nc.vector.tensor_copy(ob[:], pv[half][:])
                        nc.scalar.dma_start(out=o[col + 2 * half], in_=ob[:])
    nc.compile()
    return nc


def _get_l1():
    if "l1" not in _CACHE:
        _CACHE["l1"] = _build_l1()
    return _CACHE["l1"]


def _get_l2():
    if "l2" not in _CACHE:
        _CACHE["l2"] = _build_l2()
    return _CACHE["l2"]


def make_in_maps_l1(src, wk, wv):
    src16 = np.asarray(src).astype(FP16).reshape(B * S, D)
    wkv = np.concatenate([np.asarray(wk), np.asarray(wv)],
                         axis=1).astype(FP16)
    wkv = np.ascontiguousarray(
        wkv.reshape(8, 128, 128).transpose(1, 0, 2).reshape(128, 8 * 128))
    maps = []
    for c in CORES:
        # per-quarter SBUF image: [q][p, (j, t)] <- src[c*SR + q*512 + t,
        # j*128 + p]
        sl = src16[c * SR:(c + 1) * SR, :]                   # [2048, 1024]
        img = np.ascontiguousarray(
            sl.reshape(4, 512, 8, 128).transpose(0, 3, 2, 1)  # [q, p, j, t]
            .reshape(4, 128, 8 * 512))
        maps.append({"srcq": img, "wkv": wkv})
    return maps


def glue_l1_outputs(results):
    """Assemble kt2 / v65 from the 8 per-core kvT outputs (layout only)."""
    kvs = [np.asarray(results[c]["kvT"]) for c in CORES]
    kT_full = np.concatenate([kv[0:64] for kv in kvs], axis=1)   # [64, B*S]
    kt2 = np.ascontiguousarray(
        np.concatenate([kT_full[:, :2 * S], kT_full[:, 2 * S:]], axis=0))
    v_full = np.concatenate([kv[64:128] for kv in kvs], axis=1).T  # [B*S, 64]
    v65 = np.empty((B * S, DQ + 1), dtype=FP16)
    v65[:, :DQ] = v_full
    v65[:, DQ] = np.asarray(1.0, dtype=FP16)
    v65 = v65.reshape(B, SB, 128, DQ + 1)[[0, 2, 1, 3]]  # batch-column order
    v65 = np.ascontiguousarray(
        v65.reshape(GK, 128, DQ + 1).transpose(1, 0, 2).reshape(128, -1))
    return kt2, v65


def make_in_maps_l2(kt2, v65, tgt, mask, wq, bq, bv):
    tgt = np.asarray(tgt)
    mask = np.ascontiguousarray(mask, dtype=F32)
    wq16 = np.asarray(wq).astype(FP16)
    wq16 = np.ascontiguousarray(
        wq16.reshape(8, 128, DQ).transpose(1, 0, 2).reshape(128, 8 * DQ))
    bq = np.asarray(bq, dtype=F32)
    bq = np.ascontiguousarray(np.concatenate([bq, bq]).reshape(128, 1))
    maps = []
    for c in CORES:
        # SBUF image: [b][p, (half, j, t)]  (qproj rhs chunks, contiguous)
        tgc = tgt[:, c * TS:(c + 1) * TS, :].astype(FP16)       # [B, TS, D]
        tgi = np.ascontiguousarray(
            tgc.transpose(0, 2, 1)                              # [B, D, TS]
            .reshape(B, 8, 128, TS).transpose(0, 2, 1, 3)       # [B, p, j, t]
            .reshape(B, 128, 8 * TS))
        m = {"kt2": kt2, "v65": v65, "tgt": tgi, "wq": wq16, "bq": bq}
        masknT = mask[c * TS:(c + 1) * TS, :].T  # [S, TS]: [s, t]

        def img(sgs, arr):
            # [128, n*TS] SBUF image: col g*TS+t <- arr[sgs[g]*128+p, t]
            sub = np.stack([arr[g * 128:(g + 1) * 128] for g in sgs], axis=1)
            return np.ascontiguousarray(sub.reshape(128, len(sgs) * TS))

        if N_ASG:
            m["maskA"] = img(ACT_SGS, masknT.astype(FP16))
        if N_DSG:
            m["bmask"] = img(
                DVE_SGS, np.rint(masknT * A16 + B16C).astype(np.int16))
        if N_GSG:
            m["emask"] = img(EM_SGS, np.exp(masknT * 0.125).astype(FP16))
        maps.append(m)
    return maps


def kernel(src, tgt, mask, wq, bq, wk, bk, wv, bv):
    from concourse.bass_utils import run_bass_kernel_spmd

    res1 = run_bass_kernel_spmd(_get_l1(), make_in_maps_l1(src, wk, wv),
                                core_ids=CORES)
    kt2, v65 = glue_l1_outputs(res1.results)
    res2 = run_bass_kernel_spmd(
        _get_l2(), make_in_maps_l2(kt2, v65, tgt, mask, wq, bq, bv),
        core_ids=CORES)
    bv = np.ascontiguousarray(bv, dtype=F32)
    out = np.empty((B, S, DQ), dtype=F32)
    for c in CORES:
        oc = np.asarray(res2.results[c]["o"])          # [B, 65, TS] f32
        att = oc[:, :DQ, :] / oc[:, DQ:DQ + 1, :]      # softmax division
        out[:, c * TS:(c + 1) * TS, :] = \
            att.transpose(0, 2, 1) + bv[None, None, :]
    return out
